# revision 10
# baseline (speedup 1.0000x reference)
import numpy as np

# nn_AttentionPooling: pooled = segsum(softmax_seg(MLP(x)) * x) @ Wp + bp
# N=1M nodes, D=256, B=4096 segments, batch sorted. 8 NeuronCores.
#
# Strategy: shard nodes at segment boundaries so core c owns segments
# [512c, 512(c+1)) exactly -> the segment reduction is fully core-local and
# no collective is needed. Within a core, nodes are further split at every
# 128-segment boundary into 4 "groups"; each group accumulates a PSUM chunk
# U[128 segs, 256+1] via one-hot weighted matmuls (one-hot built on-device
# from host-precomputed relative segment ids). exp(s) is computed with a
# fixed offset C instead of the per-segment max (mathematically identical
# softmax; s is bounded by ||w2||_1 so no overflow).
#
# v2: x is shipped twice in bf16 (feature-major xtb for the MLP matmul,
# node-major xb for the pooling matmul) instead of once in f32 — same HBM
# bytes, but the on-device transpose machinery (8 PE transposes + gpsimd
# cast + pool copy per super-tile) disappears. Output tensor is fp16.

N = 1_000_000
D = 256
B = 4096
NCORES = 8
SEGS_PER_CORE = B // NCORES          # 512
CHUNK = 128                          # segments per PSUM chunk
GROUPS = SEGS_PER_CORE // CHUNK      # 4
SUB = 128                            # nodes per subtile (partition dim)
SPS = 4                              # subtiles per super-tile
C_OFF = 4.0                          # exp(s - C_OFF) for range safety

_patched = False
WORK_FRAC = 1.0  # debug knob: fraction of super-tiles emitted (timing experiments)


def _patch_drain():
    """walrus core_v3 allows 1 sync-wait per CTRL drain; split Tile's tail
    drain waits across a chain of drains."""
    global _patched
    if _patched:
        return
    import concourse.tile as tile_mod

    def _split_drain_and_barrier(self, tick_clock, wait_clock):
        drain_inst = self.nc.sync.drain()
        wait_clock.add_sem_waits(
            drain_inst.ins, tile_mod.ScopedClock({None: tick_clock.global_clock})
        )
        si = drain_inst.ins.sync_info
        if si is not None and si.on_wait is not None and len(si.on_wait) > 1:
            waits = list(si.on_wait)
            SI = type(si)
            si.on_wait = waits[:1]
            for w in waits[1:]:
                extra = self.nc.sync.drain()
                extra.ins.sync_info = SI(on_wait=[w], on_update=[])
        self.nc.all_engine_barrier()
        assert self.sems is not None
        popped = self.nc._tile_sem_poison_stack.pop()
        assert popped is self._sem_poison
        self.nc.clear_and_free_semaphores(list(self.sems.allocated().values()))
        self.nc.all_engine_barrier()

    tile_mod.TileContext._drain_and_barrier = _split_drain_and_barrier

    # Split >1-wait instructions: walrus codegen has tiny per-instruction
    # sync-wait caps. Insert same-engine NOPs carrying the excess waits.
    import concourse.mybir as mybir
    _orig_lower = tile_mod.TileContext._lower_ordered_insts

    def _lower_with_wait_split(self, ordered):
        for bbname in list(ordered.keys()):
            insts = ordered[bbname]
            newl = []
            for inst in insts:
                si = getattr(inst, "sync_info", None)
                eng = getattr(inst, "engine", None)
                ow = list(si.on_wait) if (si is not None and si.on_wait) else []
                if (
                    len(ow) > 1
                    and eng is not None
                    and eng in self.nc.engines
                    and not isinstance(inst, tile_mod.TileBranchInst)
                ):
                    SI = type(si)
                    si.on_wait = ow[-1:]
                    for w in ow[:-1]:
                        nop = self.nc.engines[eng].nop(nofuse=True, hint="wsplit")
                        nop.ins.sync_info = SI(on_wait=[w], on_update=[])
                        newl.append(nop.ins)
                newl.append(inst)
            ordered[bbname] = newl
        return _orig_lower(self, ordered)

    tile_mod.TileContext._lower_ordered_insts = _lower_with_wait_split
    _patched = True


def _build_nc(n_super_per_group, ebias_val=0.0, repeats=1):
    import concourse.bass as bass
    import concourse.mybir as mybir
    from concourse.tile import TileContext

    dt = mybir.dt
    f32 = dt.float32
    f32r = dt.float32r
    bf16 = dt.bfloat16
    f16 = dt.float16
    Alu = mybir.AluOpType
    Act = mybir.ActivationFunctionType

    SG = n_super_per_group
    n_super = GROUPS * SG
    n_sub = n_super * SPS

    nc = bass.Bass(
        target_bir_lowering=False,
        use_seq_codegen=True,
        dynamic_dma_scratch_size=65536 if repeats > 1 else 16384,
    )

    XT_W = SPS * 256          # per-super xT cols: 2 k-blocks x 512
    XB_W = SPS * 258          # per-super node-major cols: 4 x [256 x | 2 ones]

    xtb_in = nc.declare_dram_parameter("xtb", [n_super, SUB, XT_W], bf16, isOutput=False)
    xb_in = nc.declare_dram_parameter("xb", [n_super, SUB, XB_W], bf16, isOutput=False)
    relT = nc.declare_dram_parameter("relT", [SUB, n_sub], f32, isOutput=False)
    w1sb_in = nc.declare_dram_parameter("w1sb", [128, 512], bf16, isOutput=False)
    b1c_in = nc.declare_dram_parameter("b1c", [128, 2], f32, isOutput=False)
    w2c_in = nc.declare_dram_parameter("w2c", [128, 2], bf16, isOutput=False)
    wpsb_in = nc.declare_dram_parameter("wpsb", [128, 512], f32r, isOutput=False)
    bpb_in = nc.declare_dram_parameter("bpb", [128, 256], f32, isOutput=False)
    iota_in = nc.declare_dram_parameter("iota", [128, 128], f32, isOutput=False)
    ebias_in = nc.declare_dram_parameter("ebias", [128, 1], f32, isOutput=False)
    idf_in = nc.declare_dram_parameter("idf", [128, 128], f32r, isOutput=False)
    out_sh = nc.declare_dram_parameter("out", [SEGS_PER_CORE, D], f16, isOutput=True)

    from contextlib import ExitStack
    with TileContext(nc) as tc:
        with ExitStack() as stk:
            ec = stk.enter_context
            cpool = ec(tc.tile_pool(name="consts", bufs=1))
            xtpool = ec(tc.tile_pool(name="xt", bufs=8))
            xbpool = ec(tc.tile_pool(name="xb", bufs=8))
            thpool = ec(tc.tile_pool(name="th", bufs=8))
            erpool = ec(tc.tile_pool(name="erow", bufs=6))
            e4pool = ec(tc.tile_pool(name="e4", bufs=6))
            oepool = ec(tc.tile_pool(name="oe", bufs=12))
            relpool = ec(tc.tile_pool(name="rel", bufs=3))
            ufpool = ec(tc.tile_pool(name="uflush", bufs=2))
            sutpool = ec(tc.tile_pool(name="sut", bufs=2))
            rdpool = ec(tc.tile_pool(name="rd", bufs=2))
            osbpool = ec(tc.tile_pool(name="osb", bufs=2))
            # ---- constants into SBUF
            w1sb = cpool.tile([128, 512], bf16, tag="w1sb")
            nc.sync.dma_start(out=w1sb[:, :], in_=w1sb_in[:, :])
            b1c = cpool.tile([128, 2], f32, tag="b1c")
            nc.sync.dma_start(out=b1c[:, :], in_=b1c_in[:, :])
            w2c = cpool.tile([128, 2], bf16, tag="w2c")
            nc.sync.dma_start(out=w2c[:, :], in_=w2c_in[:, :])
            wpsb = cpool.tile([128, 512], f32r, tag="wpsb")
            nc.sync.dma_start(out=wpsb[:, :], in_=wpsb_in[:, :])
            bpb = cpool.tile([128, 256], f32, tag="bpb")
            nc.sync.dma_start(out=bpb[:, :], in_=bpb_in[:, :])
            iota = cpool.tile([128, 128], f32, tag="iota")
            nc.sync.dma_start(out=iota[:, :], in_=iota_in[:, :])
            idf = cpool.tile([128, 128], f32r, tag="idf")
            nc.sync.dma_start(out=idf[:, :], in_=idf_in[:, :])
            ebias = cpool.tile([128, 1], f32, tag="ebias")
            nc.sync.dma_start(out=ebias[:, :], in_=ebias_in[:, :])

            phpool = ec(tc.tile_pool(name="ph", bufs=2, space="PSUM"))
            miscpool = ec(tc.tile_pool(name="misc", bufs=2, space="PSUM"))
            pupool = ec(tc.tile_pool(name="pu", bufs=2, space="PSUM"))
            pxtpool = ec(tc.tile_pool(name="pxt", bufs=2, space="PSUM"))
            if repeats > 1:
                rep_cm = tc.For_i(0, repeats, 1)
                rep_cm.__enter__()
            rel_sb = None
            SG_EFF = max(1, int(SG * WORK_FRAC))
            for g in range(GROUPS):
                pu = pupool.tile([128, 258], f32, tag="pu")
                rel_sb = relpool.tile([128, SG * SPS], f32, tag="rel")
                nc.sync.dma_start(
                    out=rel_sb[:, :],
                    in_=relT[:, g * SG * SPS : (g + 1) * SG * SPS],
                )
                for it in range(SG_EFF):
                    sidx = g * SG + it           # super-tile index
                    xt = xtpool.tile([128, XT_W], bf16, tag="xt")
                    nc.sync.dma_start(out=xt[:, :], in_=xtb_in[sidx])
                    xb = xbpool.tile([128, XB_W], bf16, tag="xb")
                    nc.sync.dma_start(out=xb[:, :], in_=xb_in[sidx])
                    relbase = it * SPS

                    # hT = W1^T x^T  (2 dout blocks x 2 k blocks)
                    ph0 = phpool.tile([128, 512], f32, tag="ph")
                    ph1 = phpool.tile([128, 512], f32, tag="ph")
                    for dblk, ph in ((0, ph0), (1, ph1)):
                        for k in range(2):
                            nc.tensor.matmul(
                                ph[:, :],
                                lhsT=w1sb[:, (2 * k + dblk) * 128 : (2 * k + dblk + 1) * 128],
                                rhs=xt[:, k * 512 : (k + 1) * 512],
                                start=(k == 0),
                                stop=(k == 1),
                            )
                    # tanh(h + b1)  (ACT, per-partition bias)
                    th0 = thpool.tile([128, 512], bf16, tag="th0")
                    th1 = thpool.tile([128, 512], bf16, tag="th1")
                    nc.scalar.activation(th0[:, :], ph0[:, :], Act.Tanh, bias=b1c[:, 0:1])
                    nc.scalar.activation(th1[:, :], ph1[:, :], Act.Tanh, bias=b1c[:, 1:2])

                    # s = th^T w2 -> [1, 512] psum
                    misc = miscpool.tile([128, 512], f32, tag="misc")
                    ps = misc[0:1, :]
                    nc.tensor.matmul(ps, lhsT=w2c[:, 0:1], rhs=th0[:, :], start=True, stop=False)
                    nc.tensor.matmul(ps, lhsT=w2c[:, 1:2], rhs=th1[:, :], start=False, stop=True)

                    # s row -> SBUF, transpose to [128, 4], then e = exp(s + b2 - C)
                    srow = erpool.tile([1, 512], f32, tag="srow")
                    nc.vector.tensor_copy(out=srow[:, :], in_=ps)
                    pet = misc[:, 4:8]
                    for j in range(SPS):
                        nc.tensor.transpose(
                            pet[:, j : j + 1],
                            srow[0:1, j * 128 : (j + 1) * 128],
                            iota[0:1, 1:2],
                        )
                    e4 = e4pool.tile([128, 4], f32, tag="e4")
                    nc.scalar.activation(e4[:, :], pet, Act.Exp, bias=ebias[:, 0:1])

                    # per subtile: Oe = (iota==rel) * e ; U += Oe^T @ [x|1]
                    for j in range(SPS):
                        oe = oepool.tile([128, 128], bf16, tag="oe")
                        nc.vector.tensor_scalar(
                            out=oe[:, :],
                            in0=iota[:, :],
                            scalar1=rel_sb[:, relbase + j : relbase + j + 1],
                            scalar2=e4[:, j : j + 1],
                            op0=Alu.is_equal,
                            op1=Alu.mult,
                        )
                        nc.tensor.matmul(
                            pu[:, :],
                            lhsT=oe[:, :],
                            rhs=xb[:, j * 258 : j * 258 + 258],
                            start=(it == 0 and j == 0),
                            stop=(it == SG_EFF - 1 and j == SPS - 1),
                            skip_group_check=True,
                        )
                # flush group chunk to SBUF
                uf = ufpool.tile([128, 258], f32r, tag="uf")
                nc.vector.tensor_copy(out=uf[:, :], in_=pu[:, :])
                # epilogue for this group: out = (U @ Wp) / denom + bp
                put = pxtpool.tile([128, 256], f32r, tag="pxt")
                nc.tensor.transpose(put[:, 0:128], uf[:, 0:128], idf)
                nc.tensor.transpose(put[:, 128:256], uf[:, 128:256], idf)
                sut = sutpool.tile([128, 256], f32r, tag="sut")
                nc.vector.tensor_copy(out=sut[:, :], in_=put[:, :])
                po = pupool.tile([128, 256], f32, tag="pu")
                nc.tensor.matmul(po[:, :], lhsT=sut[:, 0:128], rhs=wpsb[:, 0:256], start=True, stop=False)
                nc.tensor.matmul(po[:, :], lhsT=sut[:, 128:256], rhs=wpsb[:, 256:512], start=False, stop=True)
                rd = rdpool.tile([128, 1], f32, tag="rd")
                nc.vector.reciprocal(out=rd[:, :], in_=uf[:, 256:257])
                osb = osbpool.tile([128, 256], f16, tag="osb")
                nc.vector.scalar_tensor_tensor(
                    out=osb[:, :],
                    in0=po[:, :],
                    scalar=rd[:, 0:1],
                    in1=bpb[:, :],
                    op0=Alu.mult,
                    op1=Alu.add,
                )
                nc.sync.dma_start(
                    out=out_sh[g * 128 : (g + 1) * 128, :], in_=osb[:, :]
                )
            if repeats > 1:
                rep_cm.__exit__(None, None, None)
    return nc


def _prepare(x, batch, W1, b1, w2, b2, Wp, bp):
    import ml_dtypes

    _patch_drain()

    x = np.asarray(x, dtype=np.float32)
    batch_np = np.asarray(batch).astype(np.int64)
    W1 = np.asarray(W1, dtype=np.float32)
    b1 = np.asarray(b1, dtype=np.float32)
    w2 = np.asarray(w2, dtype=np.float32)
    b2 = float(np.asarray(b2))
    Wp = np.asarray(Wp, dtype=np.float32)
    bp = np.asarray(bp, dtype=np.float32)

    n, d = x.shape
    assert (n, d) == (N, D)

    # piece p (p = 0..31): nodes whose segment is in [128p, 128(p+1))
    bounds = np.searchsorted(batch_np, np.arange(0, B + 1, CHUNK))  # [33]
    piece_nodes = np.diff(bounds)
    SG = int(np.ceil(piece_nodes.max() / (SPS * SUB)))
    n_super = GROUPS * SG
    n_sub = n_super * SPS
    n_nodes_pad = n_sub * SUB

    nc = _build_nc(SG, ebias_val=b2 - C_OFF)
    import sys
    _m = sys.modules[__name__]
    _m._last_SG = SG
    _m._last_ebias = b2 - C_OFF

    # constant payloads (shared by all cores)
    w1sb = np.zeros((128, 512), dtype=ml_dtypes.bfloat16)
    for k in range(2):
        for dblk in range(2):
            w1sb[:, (2 * k + dblk) * 128 : (2 * k + dblk + 1) * 128] = (
                W1[k * 128 : (k + 1) * 128, dblk * 128 : (dblk + 1) * 128]
            ).astype(ml_dtypes.bfloat16)
    b1c = np.stack([b1[0:128], b1[128:256]], axis=1).astype(np.float32)
    w2c = np.stack([w2[0:128], w2[128:256]], axis=1).astype(ml_dtypes.bfloat16)
    wpsb = np.zeros((128, 512), dtype=np.float32)
    wpsb[:, 0:256] = Wp[0:128, :]
    wpsb[:, 256:512] = Wp[128:256, :]
    bpb = np.tile(bp[None, :], (128, 1)).astype(np.float32)
    iota = np.tile(np.arange(128, dtype=np.float32)[None, :], (128, 1))
    idf = np.eye(128, dtype=np.float32)

    in_maps = []
    for c in range(NCORES):
        xflat = np.zeros((n_nodes_pad, D), dtype=np.float32)
        rel_c = np.full(n_sub * SUB, -1.0, dtype=np.float32)
        for g in range(GROUPS):
            p = c * GROUPS + g
            plo, phi = int(bounds[p]), int(bounds[p + 1])
            npc = phi - plo
            off = g * SG * SPS * SUB
            xflat[off : off + npc] = x[plo:phi]
            rel_c[off : off + npc] = (batch_np[plo:phi] - (p * CHUNK)).astype(
                np.float32
            )
        xbf = xflat.astype(ml_dtypes.bfloat16)
        # feature-major: [n_super, 128 part, k*512 + j*128 + n]
        xtb_c = np.ascontiguousarray(
            xbf.reshape(n_super, SPS, SUB, 2, 128).transpose(0, 4, 3, 1, 2)
            .reshape(n_super, SUB, SPS * 256)
        )
        # node-major augmented: [n_super, 128 part, j*258 + (x | 1 1)]
        xb_c = np.ones((n_super, SUB, SPS, 258), dtype=ml_dtypes.bfloat16)
        xb_c[:, :, :, 0:256] = xbf.reshape(n_super, SPS, SUB, D).transpose(
            0, 2, 1, 3
        )
        xb_c = np.ascontiguousarray(xb_c.reshape(n_super, SUB, SPS * 258))
        relT_c = np.ascontiguousarray(
            rel_c.reshape(n_sub, SUB).T
        )  # [128, n_sub]
        in_maps.append(
            {
                "xtb": xtb_c,
                "xb": xb_c,
                "relT": relT_c,
                "w1sb": w1sb,
                "b1c": b1c,
                "w2c": w2c,
                "wpsb": wpsb,
                "bpb": bpb,
                "iota": iota,
                "idf": idf,
                "ebias": np.full((128, 1), b2 - C_OFF, dtype=np.float32),
            }
        )

    return nc, in_maps


def kernel(x, batch, W1, b1, w2, b2, Wp, bp):
    from concourse.bass_utils import run_bass_kernel_spmd

    nc, in_maps = _prepare(x, batch, W1, b1, w2, b2, Wp, bp)
    import kernel as _self
    res = run_bass_kernel_spmd(nc, in_maps, core_ids=list(range(NCORES)))
    _self._last_res = res
    out = np.concatenate([res.results[c]["out"] for c in range(NCORES)], axis=0)
    return out.astype(np.float32)


# revision 14
# speedup vs baseline: 1.5497x; 1.5497x over previous
import numpy as np

# nn_AttentionPooling: pooled = segsum(softmax_seg(MLP(x)) * x) @ Wp + bp
# N=1M nodes, D=256, B=4096 segments, batch sorted. 8 NeuronCores.
#
# Strategy: shard nodes at segment boundaries so core c owns segments
# [512c, 512(c+1)) exactly -> the segment reduction is fully core-local and
# no collective is needed. Within a core, nodes are further split at every
# 128-segment boundary into 4 "groups"; each group accumulates a PSUM chunk
# U[128 segs, 256+1] via one-hot weighted matmuls (one-hot built on-device
# from host-precomputed relative segment ids). exp(s) is computed with a
# fixed offset C instead of the per-segment max (mathematically identical
# softmax; s is bounded by ||w2||_1 so no overflow).
#
# v2: x is shipped twice in bf16 (feature-major xtb for the MLP matmul,
# node-major xb for the pooling matmul) instead of once in f32 — same HBM
# bytes, but the on-device transpose machinery (8 PE transposes + gpsimd
# cast + pool copy per super-tile) disappears. Output tensor is fp16.

N = 1_000_000
D = 256
B = 4096
NCORES = 8
SEGS_PER_CORE = B // NCORES          # 512
CHUNK = 128                          # segments per PSUM chunk
GROUPS = SEGS_PER_CORE // CHUNK      # 4
SUB = 128                            # nodes per subtile (partition dim)
SPS = 4                              # subtiles per super-tile
C_OFF = 4.0                          # exp(s - C_OFF) for range safety

_patched = False
WORK_FRAC = 1.0  # debug knob: fraction of super-tiles emitted (timing experiments)
POOL_DEEP = True  # debug knob: deeper SBUF pools
XB_FRAC = 1.0    # debug knob: fraction of xb cols DMA'd (timing experiments only)


def _patch_drain():
    """walrus core_v3 allows 1 sync-wait per CTRL drain; split Tile's tail
    drain waits across a chain of drains."""
    global _patched
    if _patched:
        return
    import concourse.tile as tile_mod

    def _split_drain_and_barrier(self, tick_clock, wait_clock):
        drain_inst = self.nc.sync.drain()
        wait_clock.add_sem_waits(
            drain_inst.ins, tile_mod.ScopedClock({None: tick_clock.global_clock})
        )
        si = drain_inst.ins.sync_info
        if si is not None and si.on_wait is not None and len(si.on_wait) > 1:
            waits = list(si.on_wait)
            SI = type(si)
            si.on_wait = waits[:1]
            for w in waits[1:]:
                extra = self.nc.sync.drain()
                extra.ins.sync_info = SI(on_wait=[w], on_update=[])
        self.nc.all_engine_barrier()
        assert self.sems is not None
        popped = self.nc._tile_sem_poison_stack.pop()
        assert popped is self._sem_poison
        self.nc.clear_and_free_semaphores(list(self.sems.allocated().values()))
        self.nc.all_engine_barrier()

    tile_mod.TileContext._drain_and_barrier = _split_drain_and_barrier

    # Split >1-wait instructions: walrus codegen has tiny per-instruction
    # sync-wait caps. Insert same-engine NOPs carrying the excess waits.
    import concourse.mybir as mybir
    _orig_lower = tile_mod.TileContext._lower_ordered_insts

    def _lower_with_wait_split(self, ordered):
        for bbname in list(ordered.keys()):
            insts = ordered[bbname]
            newl = []
            for inst in insts:
                si = getattr(inst, "sync_info", None)
                eng = getattr(inst, "engine", None)
                ow = list(si.on_wait) if (si is not None and si.on_wait) else []
                if (
                    len(ow) > 1
                    and eng is not None
                    and eng in self.nc.engines
                    and not isinstance(inst, tile_mod.TileBranchInst)
                ):
                    SI = type(si)
                    si.on_wait = ow[-1:]
                    for w in ow[:-1]:
                        nop = self.nc.engines[eng].nop(nofuse=True, hint="wsplit")
                        nop.ins.sync_info = SI(on_wait=[w], on_update=[])
                        newl.append(nop.ins)
                newl.append(inst)
            ordered[bbname] = newl
        return _orig_lower(self, ordered)

    tile_mod.TileContext._lower_ordered_insts = _lower_with_wait_split
    _patched = True


def _build_nc(n_super_per_group, ebias_val=0.0, repeats=1):
    import concourse.bass as bass
    import concourse.mybir as mybir
    from concourse.tile import TileContext

    dt = mybir.dt
    f32 = dt.float32
    f32r = dt.float32r
    bf16 = dt.bfloat16
    f16 = dt.float16
    Alu = mybir.AluOpType
    Act = mybir.ActivationFunctionType

    SG = n_super_per_group
    n_super = GROUPS * SG
    n_sub = n_super * SPS

    nc = bass.Bass(
        target_bir_lowering=False,
        use_seq_codegen=True,
        dynamic_dma_scratch_size=65536 if repeats > 1 else 16384,
    )

    XT_W = SPS * 256          # per-super xT cols: 2 k-blocks x 512
    XB_W = SPS * 258          # per-super node-major cols: 4 x [256 x | 2 ones]

    xtb_in = nc.declare_dram_parameter("xtb", [n_super, SUB, XT_W], bf16, isOutput=False)
    xb_in = nc.declare_dram_parameter("xb", [n_super, SUB, XB_W], bf16, isOutput=False)
    relT = nc.declare_dram_parameter("relT", [SUB, n_sub], f32, isOutput=False)
    w1sb_in = nc.declare_dram_parameter("w1sb", [128, 512], bf16, isOutput=False)
    b1c_in = nc.declare_dram_parameter("b1c", [128, 2], f32, isOutput=False)
    w2c_in = nc.declare_dram_parameter("w2c", [128, 2], bf16, isOutput=False)
    wpsb_in = nc.declare_dram_parameter("wpsb", [128, 512], f32r, isOutput=False)
    bpb_in = nc.declare_dram_parameter("bpb", [128, 256], f32, isOutput=False)
    iota_in = nc.declare_dram_parameter("iota", [128, 128], f32, isOutput=False)
    ebias_in = nc.declare_dram_parameter("ebias", [128, 1], f32, isOutput=False)
    idf_in = nc.declare_dram_parameter("idf", [128, 128], f32r, isOutput=False)
    out_sh = nc.declare_dram_parameter("out", [SEGS_PER_CORE, D], f16, isOutput=True)

    from contextlib import ExitStack
    with TileContext(nc) as tc:
        with ExitStack() as stk:
            ec = stk.enter_context
            cpool = ec(tc.tile_pool(name="consts", bufs=1))
            dp = POOL_DEEP
            xtpool = ec(tc.tile_pool(name="xt", bufs=12 if dp else 8))
            xbpool = ec(tc.tile_pool(name="xb", bufs=12 if dp else 8))
            thpool = ec(tc.tile_pool(name="th", bufs=12 if dp else 8))
            erpool = ec(tc.tile_pool(name="erow", bufs=10 if dp else 6))
            e4pool = ec(tc.tile_pool(name="e4", bufs=10 if dp else 6))
            oepool = ec(tc.tile_pool(name="oe", bufs=20 if dp else 12))
            relpool = ec(tc.tile_pool(name="rel", bufs=3))
            ufpool = ec(tc.tile_pool(name="uflush", bufs=2))
            sutpool = ec(tc.tile_pool(name="sut", bufs=2))
            rdpool = ec(tc.tile_pool(name="rd", bufs=2))
            osbpool = ec(tc.tile_pool(name="osb", bufs=2))
            # ---- constants into SBUF
            w1sb = cpool.tile([128, 512], bf16, tag="w1sb")
            nc.sync.dma_start(out=w1sb[:, :], in_=w1sb_in[:, :])
            b1c = cpool.tile([128, 2], f32, tag="b1c")
            nc.sync.dma_start(out=b1c[:, :], in_=b1c_in[:, :])
            w2c = cpool.tile([128, 2], bf16, tag="w2c")
            nc.sync.dma_start(out=w2c[:, :], in_=w2c_in[:, :])
            wpsb = cpool.tile([128, 512], f32r, tag="wpsb")
            nc.sync.dma_start(out=wpsb[:, :], in_=wpsb_in[:, :])
            bpb = cpool.tile([128, 256], f32, tag="bpb")
            nc.sync.dma_start(out=bpb[:, :], in_=bpb_in[:, :])
            iota = cpool.tile([128, 128], f32, tag="iota")
            nc.sync.dma_start(out=iota[:, :], in_=iota_in[:, :])
            idf = cpool.tile([128, 128], f32r, tag="idf")
            nc.sync.dma_start(out=idf[:, :], in_=idf_in[:, :])
            ebias = cpool.tile([128, 1], f32, tag="ebias")
            nc.sync.dma_start(out=ebias[:, :], in_=ebias_in[:, :])

            phpool = ec(tc.tile_pool(name="ph", bufs=4, space="PSUM"))
            miscpool = ec(tc.tile_pool(name="misc", bufs=2, space="PSUM"))
            pupool = ec(tc.tile_pool(name="pu", bufs=2, space="PSUM"))
            if repeats > 1:
                rep_cm = tc.For_i(0, repeats, 1)
                rep_cm.__enter__()
            rel_sb = None
            SG_EFF = max(1, int(SG * WORK_FRAC))
            for g in range(GROUPS):
                pu = pupool.tile([128, 258], f32, tag="pu")
                rel_sb = relpool.tile([128, SG * SPS], f32, tag="rel")
                nc.sync.dma_start(
                    out=rel_sb[:, :],
                    in_=relT[:, g * SG * SPS : (g + 1) * SG * SPS],
                )

                # stage emitters: two supers are interleaved stage-by-stage so
                # every engine has independent work between dependent steps
                st = {}

                def emit_A(it):  # DMA + hT = W1^T x^T
                    sidx = g * SG + it
                    xt = xtpool.tile([128, XT_W], bf16, tag="xt")
                    nc.sync.dma_start(out=xt[:, :], in_=xtb_in[sidx])
                    xb = xbpool.tile([128, XB_W], bf16, tag="xb")
                    if XB_FRAC >= 1.0:
                        nc.sync.dma_start(out=xb[:, :], in_=xb_in[sidx])
                    else:
                        xbc = int(XB_W * XB_FRAC)
                        nc.sync.dma_start(out=xb[:, 0:xbc], in_=xb_in[sidx][:, 0:xbc])
                    ph0 = phpool.tile([128, 512], f32, tag="ph")
                    ph1 = phpool.tile([128, 512], f32, tag="ph")
                    for dblk, ph in ((0, ph0), (1, ph1)):
                        for k in range(2):
                            nc.tensor.matmul(
                                ph[:, :],
                                lhsT=w1sb[:, (2 * k + dblk) * 128 : (2 * k + dblk + 1) * 128],
                                rhs=xt[:, k * 512 : (k + 1) * 512],
                                start=(k == 0),
                                stop=(k == 1),
                            )
                    st[it] = {"xb": xb, "ph0": ph0, "ph1": ph1}

                def emit_B(it):  # tanh(h + b1)
                    s = st[it]
                    th0 = thpool.tile([128, 512], bf16, tag="th0")
                    th1 = thpool.tile([128, 512], bf16, tag="th1")
                    nc.scalar.activation(th0[:, :], s["ph0"][:, :], Act.Tanh, bias=b1c[:, 0:1])
                    nc.scalar.activation(th1[:, :], s["ph1"][:, :], Act.Tanh, bias=b1c[:, 1:2])
                    s["th0"], s["th1"] = th0, th1

                def emit_C(it):  # s = th^T w2 -> [1, 512] psum
                    s = st[it]
                    misc = miscpool.tile([128, 512], f32, tag="misc")
                    ps = misc[0:1, :]
                    nc.tensor.matmul(ps, lhsT=w2c[:, 0:1], rhs=s["th0"][:, :], start=True, stop=False)
                    nc.tensor.matmul(ps, lhsT=w2c[:, 1:2], rhs=s["th1"][:, :], start=False, stop=True)
                    s["misc"], s["ps"] = misc, ps

                def emit_D(it):  # s row -> SBUF
                    s = st[it]
                    srow = erpool.tile([1, 512], f32, tag="srow")
                    nc.vector.tensor_copy(out=srow[:, :], in_=s["ps"])
                    s["srow"] = srow

                def emit_E(it):  # transpose s -> [128, 4] psum
                    s = st[it]
                    pet = s["misc"][:, 4:8]
                    for j in range(SPS):
                        nc.tensor.transpose(
                            pet[:, j : j + 1],
                            s["srow"][0:1, j * 128 : (j + 1) * 128],
                            iota[0:1, 1:2],
                        )
                    s["pet"] = pet

                def emit_F(it):  # e = exp(s + b2 - C) -> [128, 4] sbuf
                    s = st[it]
                    e4 = e4pool.tile([128, 4], f32, tag="e4")
                    nc.scalar.activation(e4[:, :], s["pet"], Act.Exp, bias=ebias[:, 0:1])
                    s["e4"] = e4

                def emit_G(it):  # Oe_j = (iota==rel_j) * e_j
                    s = st[it]
                    relbase = it * SPS
                    oes = []
                    for j in range(SPS):
                        oe = oepool.tile([128, 128], bf16, tag="oe")
                        nc.vector.tensor_scalar(
                            out=oe[:, :],
                            in0=iota[:, :],
                            scalar1=rel_sb[:, relbase + j : relbase + j + 1],
                            scalar2=s["e4"][:, j : j + 1],
                            op0=Alu.is_equal,
                            op1=Alu.mult,
                        )
                        oes.append(oe)
                    s["oes"] = oes

                def emit_H(it):  # U += Oe^T @ [x|1]
                    s = st.pop(it)
                    for j in range(SPS):
                        nc.tensor.matmul(
                            pu[:, :],
                            lhsT=s["oes"][j][:, :],
                            rhs=s["xb"][:, j * 258 : j * 258 + 258],
                            start=(it == 0 and j == 0),
                            stop=(it == SG_EFF - 1 and j == SPS - 1),
                            skip_group_check=True,
                        )

                stages = (emit_A, emit_B, emit_C, emit_D, emit_E, emit_F, emit_G, emit_H)
                for it0 in range(0, SG_EFF, 2):
                    pair = [it0] + ([it0 + 1] if it0 + 1 < SG_EFF else [])
                    for emit in stages:
                        for it in pair:
                            emit(it)
                # flush group chunk to SBUF
                uf = ufpool.tile([128, 258], f32r, tag="uf")
                nc.vector.tensor_copy(out=uf[:, :], in_=pu[:, :])
                # epilogue for this group: out = (U @ Wp) / denom + bp
                putm = miscpool.tile([128, 512], f32r, tag="misc")
                put = putm[:, 0:256]
                nc.tensor.transpose(put[:, 0:128], uf[:, 0:128], idf)
                nc.tensor.transpose(put[:, 128:256], uf[:, 128:256], idf)
                sut = sutpool.tile([128, 256], f32r, tag="sut")
                nc.vector.tensor_copy(out=sut[:, :], in_=put[:, :])
                po = pupool.tile([128, 256], f32, tag="pu")
                nc.tensor.matmul(po[:, :], lhsT=sut[:, 0:128], rhs=wpsb[:, 0:256], start=True, stop=False)
                nc.tensor.matmul(po[:, :], lhsT=sut[:, 128:256], rhs=wpsb[:, 256:512], start=False, stop=True)
                rd = rdpool.tile([128, 1], f32, tag="rd")
                nc.vector.reciprocal(out=rd[:, :], in_=uf[:, 256:257])
                osb = osbpool.tile([128, 256], f16, tag="osb")
                nc.vector.scalar_tensor_tensor(
                    out=osb[:, :],
                    in0=po[:, :],
                    scalar=rd[:, 0:1],
                    in1=bpb[:, :],
                    op0=Alu.mult,
                    op1=Alu.add,
                )
                nc.sync.dma_start(
                    out=out_sh[g * 128 : (g + 1) * 128, :], in_=osb[:, :]
                )
            if repeats > 1:
                rep_cm.__exit__(None, None, None)
    return nc


def _prepare(x, batch, W1, b1, w2, b2, Wp, bp):
    import ml_dtypes

    _patch_drain()

    x = np.asarray(x, dtype=np.float32)
    batch_np = np.asarray(batch).astype(np.int64)
    W1 = np.asarray(W1, dtype=np.float32)
    b1 = np.asarray(b1, dtype=np.float32)
    w2 = np.asarray(w2, dtype=np.float32)
    b2 = float(np.asarray(b2))
    Wp = np.asarray(Wp, dtype=np.float32)
    bp = np.asarray(bp, dtype=np.float32)

    n, d = x.shape
    assert (n, d) == (N, D)

    # piece p (p = 0..31): nodes whose segment is in [128p, 128(p+1))
    bounds = np.searchsorted(batch_np, np.arange(0, B + 1, CHUNK))  # [33]
    piece_nodes = np.diff(bounds)
    SG = int(np.ceil(piece_nodes.max() / (SPS * SUB)))
    n_super = GROUPS * SG
    n_sub = n_super * SPS
    n_nodes_pad = n_sub * SUB

    nc = _build_nc(SG, ebias_val=b2 - C_OFF)
    import sys
    _m = sys.modules[__name__]
    _m._last_SG = SG
    _m._last_ebias = b2 - C_OFF

    # constant payloads (shared by all cores)
    w1sb = np.zeros((128, 512), dtype=ml_dtypes.bfloat16)
    for k in range(2):
        for dblk in range(2):
            w1sb[:, (2 * k + dblk) * 128 : (2 * k + dblk + 1) * 128] = (
                W1[k * 128 : (k + 1) * 128, dblk * 128 : (dblk + 1) * 128]
            ).astype(ml_dtypes.bfloat16)
    b1c = np.stack([b1[0:128], b1[128:256]], axis=1).astype(np.float32)
    w2c = np.stack([w2[0:128], w2[128:256]], axis=1).astype(ml_dtypes.bfloat16)
    wpsb = np.zeros((128, 512), dtype=np.float32)
    wpsb[:, 0:256] = Wp[0:128, :]
    wpsb[:, 256:512] = Wp[128:256, :]
    bpb = np.tile(bp[None, :], (128, 1)).astype(np.float32)
    iota = np.tile(np.arange(128, dtype=np.float32)[None, :], (128, 1))
    idf = np.eye(128, dtype=np.float32)

    in_maps = []
    for c in range(NCORES):
        xflat = np.zeros((n_nodes_pad, D), dtype=np.float32)
        rel_c = np.full(n_sub * SUB, -1.0, dtype=np.float32)
        for g in range(GROUPS):
            p = c * GROUPS + g
            plo, phi = int(bounds[p]), int(bounds[p + 1])
            npc = phi - plo
            off = g * SG * SPS * SUB
            xflat[off : off + npc] = x[plo:phi]
            rel_c[off : off + npc] = (batch_np[plo:phi] - (p * CHUNK)).astype(
                np.float32
            )
        xbf = xflat.astype(ml_dtypes.bfloat16)
        # feature-major: [n_super, 128 part, k*512 + j*128 + n]
        xtb_c = np.ascontiguousarray(
            xbf.reshape(n_super, SPS, SUB, 2, 128).transpose(0, 4, 3, 1, 2)
            .reshape(n_super, SUB, SPS * 256)
        )
        # node-major augmented: [n_super, 128 part, j*258 + (x | 1 1)]
        xb_c = np.ones((n_super, SUB, SPS, 258), dtype=ml_dtypes.bfloat16)
        xb_c[:, :, :, 0:256] = xbf.reshape(n_super, SPS, SUB, D).transpose(
            0, 2, 1, 3
        )
        xb_c = np.ascontiguousarray(xb_c.reshape(n_super, SUB, SPS * 258))
        relT_c = np.ascontiguousarray(
            rel_c.reshape(n_sub, SUB).T
        )  # [128, n_sub]
        in_maps.append(
            {
                "xtb": xtb_c,
                "xb": xb_c,
                "relT": relT_c,
                "w1sb": w1sb,
                "b1c": b1c,
                "w2c": w2c,
                "wpsb": wpsb,
                "bpb": bpb,
                "iota": iota,
                "idf": idf,
                "ebias": np.full((128, 1), b2 - C_OFF, dtype=np.float32),
            }
        )

    return nc, in_maps


def kernel(x, batch, W1, b1, w2, b2, Wp, bp):
    from concourse.bass_utils import run_bass_kernel_spmd

    nc, in_maps = _prepare(x, batch, W1, b1, w2, b2, Wp, bp)
    import kernel as _self
    res = run_bass_kernel_spmd(nc, in_maps, core_ids=list(range(NCORES)))
    _self._last_res = res
    out = np.concatenate([res.results[c]["out"] for c in range(NCORES)], axis=0)
    return out.astype(np.float32)


# revision 15
# speedup vs baseline: 1.7593x; 1.1352x over previous
import numpy as np

# nn_AttentionPooling: pooled = segsum(softmax_seg(MLP(x)) * x) @ Wp + bp
# N=1M nodes, D=256, B=4096 segments, batch sorted. 8 NeuronCores.
#
# Strategy: shard nodes at segment boundaries so core c owns segments
# [512c, 512(c+1)) exactly -> the segment reduction is fully core-local and
# no collective is needed. Within a core, nodes are further split at every
# 128-segment boundary into 4 "groups"; each group accumulates a PSUM chunk
# U[128 segs, 256+1] via one-hot weighted matmuls (one-hot built on-device
# from host-precomputed relative segment ids). exp(s) is computed with a
# fixed offset C instead of the per-segment max (mathematically identical
# softmax; s is bounded by ||w2||_1 so no overflow).
#
# v2: x is shipped twice in bf16 (feature-major xtb for the MLP matmul,
# node-major xb for the pooling matmul) instead of once in f32 — same HBM
# bytes, but the on-device transpose machinery (8 PE transposes + gpsimd
# cast + pool copy per super-tile) disappears. Output tensor is fp16.

N = 1_000_000
D = 256
B = 4096
NCORES = 8
SEGS_PER_CORE = B // NCORES          # 512
CHUNK = 128                          # segments per PSUM chunk
GROUPS = SEGS_PER_CORE // CHUNK      # 4
SUB = 128                            # nodes per subtile (partition dim)
SPS = 4                              # subtiles per super-tile
C_OFF = 4.0                          # exp(s - C_OFF) for range safety

_patched = False
WORK_FRAC = 1.0  # debug knob: fraction of super-tiles emitted (timing experiments)
POOL_DEEP = True  # debug knob: deeper SBUF pools
H_FP8 = True     # MLP (h = W1^T x) matmul in fp8e4 DoubleRow (2 k-tiles/pass)
XB_FRAC = 1.0    # debug knob: fraction of xb cols DMA'd (timing experiments only)


def _patch_drain():
    """walrus core_v3 allows 1 sync-wait per CTRL drain; split Tile's tail
    drain waits across a chain of drains."""
    global _patched
    if _patched:
        return
    import concourse.tile as tile_mod

    def _split_drain_and_barrier(self, tick_clock, wait_clock):
        drain_inst = self.nc.sync.drain()
        wait_clock.add_sem_waits(
            drain_inst.ins, tile_mod.ScopedClock({None: tick_clock.global_clock})
        )
        si = drain_inst.ins.sync_info
        if si is not None and si.on_wait is not None and len(si.on_wait) > 1:
            waits = list(si.on_wait)
            SI = type(si)
            si.on_wait = waits[:1]
            for w in waits[1:]:
                extra = self.nc.sync.drain()
                extra.ins.sync_info = SI(on_wait=[w], on_update=[])
        self.nc.all_engine_barrier()
        assert self.sems is not None
        popped = self.nc._tile_sem_poison_stack.pop()
        assert popped is self._sem_poison
        self.nc.clear_and_free_semaphores(list(self.sems.allocated().values()))
        self.nc.all_engine_barrier()

    tile_mod.TileContext._drain_and_barrier = _split_drain_and_barrier

    # Split >1-wait instructions: walrus codegen has tiny per-instruction
    # sync-wait caps. Insert same-engine NOPs carrying the excess waits.
    import concourse.mybir as mybir
    _orig_lower = tile_mod.TileContext._lower_ordered_insts

    def _lower_with_wait_split(self, ordered):
        for bbname in list(ordered.keys()):
            insts = ordered[bbname]
            newl = []
            for inst in insts:
                si = getattr(inst, "sync_info", None)
                eng = getattr(inst, "engine", None)
                ow = list(si.on_wait) if (si is not None and si.on_wait) else []
                if (
                    len(ow) > 1
                    and eng is not None
                    and eng in self.nc.engines
                    and not isinstance(inst, tile_mod.TileBranchInst)
                ):
                    SI = type(si)
                    si.on_wait = ow[-1:]
                    for w in ow[:-1]:
                        nop = self.nc.engines[eng].nop(nofuse=True, hint="wsplit")
                        nop.ins.sync_info = SI(on_wait=[w], on_update=[])
                        newl.append(nop.ins)
                newl.append(inst)
            ordered[bbname] = newl
        return _orig_lower(self, ordered)

    tile_mod.TileContext._lower_ordered_insts = _lower_with_wait_split
    _patched = True


def _build_nc(n_super_per_group, ebias_val=0.0, repeats=1):
    import concourse.bass as bass
    import concourse.mybir as mybir
    from concourse.tile import TileContext

    dt = mybir.dt
    f32 = dt.float32
    f32r = dt.float32r
    bf16 = dt.bfloat16
    f16 = dt.float16
    Alu = mybir.AluOpType
    Act = mybir.ActivationFunctionType

    SG = n_super_per_group
    n_super = GROUPS * SG
    n_sub = n_super * SPS

    nc = bass.Bass(
        target_bir_lowering=False,
        use_seq_codegen=True,
        dynamic_dma_scratch_size=65536 if repeats > 1 else 16384,
    )

    XT_W = SPS * 256          # per-super xT cols: 2 k-blocks x 512
    XB_W = SPS * 258          # per-super node-major cols: 4 x [256 x | 2 ones]

    f8 = dt.float8e4
    xt_dt = f8 if H_FP8 else bf16
    xtb_in = nc.declare_dram_parameter("xtb", [n_super, SUB, XT_W], xt_dt, isOutput=False)
    xb_in = nc.declare_dram_parameter("xb", [n_super, SUB, XB_W], bf16, isOutput=False)
    relT = nc.declare_dram_parameter("relT", [SUB, n_sub], f32, isOutput=False)
    w1sb_in = nc.declare_dram_parameter("w1sb", [128, 512], xt_dt, isOutput=False)
    b1c_in = nc.declare_dram_parameter("b1c", [128, 2], f32, isOutput=False)
    w2c_in = nc.declare_dram_parameter("w2c", [128, 2], bf16, isOutput=False)
    wpsb_in = nc.declare_dram_parameter("wpsb", [128, 512], f32r, isOutput=False)
    bpb_in = nc.declare_dram_parameter("bpb", [128, 256], f32, isOutput=False)
    iota_in = nc.declare_dram_parameter("iota", [128, 128], f32, isOutput=False)
    ebias_in = nc.declare_dram_parameter("ebias", [128, 1], f32, isOutput=False)
    idf_in = nc.declare_dram_parameter("idf", [128, 128], f32r, isOutput=False)
    out_sh = nc.declare_dram_parameter("out", [SEGS_PER_CORE, D], f16, isOutput=True)

    from contextlib import ExitStack
    with TileContext(nc) as tc:
        with ExitStack() as stk:
            ec = stk.enter_context
            cpool = ec(tc.tile_pool(name="consts", bufs=1))
            dp = POOL_DEEP
            xtpool = ec(tc.tile_pool(name="xt", bufs=12 if dp else 8))
            xbpool = ec(tc.tile_pool(name="xb", bufs=12 if dp else 8))
            thpool = ec(tc.tile_pool(name="th", bufs=12 if dp else 8))
            erpool = ec(tc.tile_pool(name="erow", bufs=10 if dp else 6))
            e4pool = ec(tc.tile_pool(name="e4", bufs=10 if dp else 6))
            oepool = ec(tc.tile_pool(name="oe", bufs=20 if dp else 12))
            relpool = ec(tc.tile_pool(name="rel", bufs=3))
            ufpool = ec(tc.tile_pool(name="uflush", bufs=2))
            sutpool = ec(tc.tile_pool(name="sut", bufs=2))
            rdpool = ec(tc.tile_pool(name="rd", bufs=2))
            osbpool = ec(tc.tile_pool(name="osb", bufs=2))
            # ---- constants into SBUF
            w1sb = cpool.tile([128, 512], xt_dt, tag="w1sb")
            nc.sync.dma_start(out=w1sb[:, :], in_=w1sb_in[:, :])
            b1c = cpool.tile([128, 2], f32, tag="b1c")
            nc.sync.dma_start(out=b1c[:, :], in_=b1c_in[:, :])
            w2c = cpool.tile([128, 2], bf16, tag="w2c")
            nc.sync.dma_start(out=w2c[:, :], in_=w2c_in[:, :])
            wpsb = cpool.tile([128, 512], f32r, tag="wpsb")
            nc.sync.dma_start(out=wpsb[:, :], in_=wpsb_in[:, :])
            bpb = cpool.tile([128, 256], f32, tag="bpb")
            nc.sync.dma_start(out=bpb[:, :], in_=bpb_in[:, :])
            iota = cpool.tile([128, 128], f32, tag="iota")
            nc.sync.dma_start(out=iota[:, :], in_=iota_in[:, :])
            idf = cpool.tile([128, 128], f32r, tag="idf")
            nc.sync.dma_start(out=idf[:, :], in_=idf_in[:, :])
            ebias = cpool.tile([128, 1], f32, tag="ebias")
            nc.sync.dma_start(out=ebias[:, :], in_=ebias_in[:, :])

            phpool = ec(tc.tile_pool(name="ph", bufs=4, space="PSUM"))
            miscpool = ec(tc.tile_pool(name="misc", bufs=2, space="PSUM"))
            pupool = ec(tc.tile_pool(name="pu", bufs=2, space="PSUM"))
            if repeats > 1:
                rep_cm = tc.For_i(0, repeats, 1)
                rep_cm.__enter__()
            rel_sb = None
            SG_EFF = max(1, int(SG * WORK_FRAC))
            for g in range(GROUPS):
                pu = pupool.tile([128, 258], f32, tag="pu")
                rel_sb = relpool.tile([128, SG * SPS], f32, tag="rel")
                nc.sync.dma_start(
                    out=rel_sb[:, :],
                    in_=relT[:, g * SG * SPS : (g + 1) * SG * SPS],
                )

                # stage emitters: two supers are interleaved stage-by-stage so
                # every engine has independent work between dependent steps
                st = {}

                def emit_A(it):  # DMA + hT = W1^T x^T
                    sidx = g * SG + it
                    xt = xtpool.tile([128, XT_W], xt_dt, tag="xt")
                    nc.sync.dma_start(out=xt[:, :], in_=xtb_in[sidx])
                    xb = xbpool.tile([128, XB_W], bf16, tag="xb")
                    if XB_FRAC >= 1.0:
                        nc.sync.dma_start(out=xb[:, :], in_=xb_in[sidx])
                    else:
                        xbc = int(XB_W * XB_FRAC)
                        nc.sync.dma_start(out=xb[:, 0:xbc], in_=xb_in[sidx][:, 0:xbc])
                    ph0 = phpool.tile([128, 512], f32, tag="ph")
                    ph1 = phpool.tile([128, 512], f32, tag="ph")
                    if H_FP8:
                        # one DoubleRow matmul per dout block: both 128-row
                        # k-tiles are contracted in a single pass
                        w1r = w1sb.rearrange("p (k d c) -> p k d c", k=2, d=2)
                        xtr = xt.rearrange("p (k c) -> p k c", k=2)
                        for dblk, ph in ((0, ph0), (1, ph1)):
                            nc.tensor.matmul(
                                ph[:, :],
                                lhsT=w1r[:, :, dblk, :],
                                rhs=xtr[:, :, :],
                                start=True,
                                stop=True,
                                perf_mode=mybir.MatmulPerfMode.DoubleRow,
                            )
                    else:
                        for dblk, ph in ((0, ph0), (1, ph1)):
                            for k in range(2):
                                nc.tensor.matmul(
                                    ph[:, :],
                                    lhsT=w1sb[:, (2 * k + dblk) * 128 : (2 * k + dblk + 1) * 128],
                                    rhs=xt[:, k * 512 : (k + 1) * 512],
                                    start=(k == 0),
                                    stop=(k == 1),
                                )
                    st[it] = {"xb": xb, "ph0": ph0, "ph1": ph1}

                def emit_B(it):  # tanh(h + b1)
                    s = st[it]
                    th0 = thpool.tile([128, 512], bf16, tag="th0")
                    th1 = thpool.tile([128, 512], bf16, tag="th1")
                    nc.scalar.activation(th0[:, :], s["ph0"][:, :], Act.Tanh, bias=b1c[:, 0:1])
                    nc.scalar.activation(th1[:, :], s["ph1"][:, :], Act.Tanh, bias=b1c[:, 1:2])
                    s["th0"], s["th1"] = th0, th1

                def emit_C(it):  # s = th^T w2 -> [1, 512] psum
                    s = st[it]
                    misc = miscpool.tile([128, 512], f32, tag="misc")
                    ps = misc[0:1, :]
                    nc.tensor.matmul(ps, lhsT=w2c[:, 0:1], rhs=s["th0"][:, :], start=True, stop=False)
                    nc.tensor.matmul(ps, lhsT=w2c[:, 1:2], rhs=s["th1"][:, :], start=False, stop=True)
                    s["misc"], s["ps"] = misc, ps

                def emit_D(it):  # s row -> SBUF
                    s = st[it]
                    srow = erpool.tile([1, 512], f32, tag="srow")
                    nc.vector.tensor_copy(out=srow[:, :], in_=s["ps"])
                    s["srow"] = srow

                def emit_E(it):  # transpose s -> [128, 4] psum
                    s = st[it]
                    pet = s["misc"][:, 4:8]
                    for j in range(SPS):
                        nc.tensor.transpose(
                            pet[:, j : j + 1],
                            s["srow"][0:1, j * 128 : (j + 1) * 128],
                            iota[0:1, 1:2],
                        )
                    s["pet"] = pet

                def emit_F(it):  # e = exp(s + b2 - C) -> [128, 4] sbuf
                    s = st[it]
                    e4 = e4pool.tile([128, 4], f32, tag="e4")
                    nc.scalar.activation(e4[:, :], s["pet"], Act.Exp, bias=ebias[:, 0:1])
                    s["e4"] = e4

                def emit_G(it):  # Oe_j = (iota==rel_j) * e_j
                    s = st[it]
                    relbase = it * SPS
                    oes = []
                    for j in range(SPS):
                        oe = oepool.tile([128, 128], bf16, tag="oe")
                        nc.vector.tensor_scalar(
                            out=oe[:, :],
                            in0=iota[:, :],
                            scalar1=rel_sb[:, relbase + j : relbase + j + 1],
                            scalar2=s["e4"][:, j : j + 1],
                            op0=Alu.is_equal,
                            op1=Alu.mult,
                        )
                        oes.append(oe)
                    s["oes"] = oes

                def emit_H(it):  # U += Oe^T @ [x|1]
                    s = st.pop(it)
                    for j in range(SPS):
                        nc.tensor.matmul(
                            pu[:, :],
                            lhsT=s["oes"][j][:, :],
                            rhs=s["xb"][:, j * 258 : j * 258 + 258],
                            start=(it == 0 and j == 0),
                            stop=(it == SG_EFF - 1 and j == SPS - 1),
                            skip_group_check=True,
                        )

                stages = (emit_A, emit_B, emit_C, emit_D, emit_E, emit_F, emit_G, emit_H)
                for it0 in range(0, SG_EFF, 2):
                    pair = [it0] + ([it0 + 1] if it0 + 1 < SG_EFF else [])
                    for emit in stages:
                        for it in pair:
                            emit(it)
                # flush group chunk to SBUF
                uf = ufpool.tile([128, 258], f32r, tag="uf")
                nc.vector.tensor_copy(out=uf[:, :], in_=pu[:, :])
                # epilogue for this group: out = (U @ Wp) / denom + bp
                putm = miscpool.tile([128, 512], f32r, tag="misc")
                put = putm[:, 0:256]
                nc.tensor.transpose(put[:, 0:128], uf[:, 0:128], idf)
                nc.tensor.transpose(put[:, 128:256], uf[:, 128:256], idf)
                sut = sutpool.tile([128, 256], f32r, tag="sut")
                nc.vector.tensor_copy(out=sut[:, :], in_=put[:, :])
                po = pupool.tile([128, 256], f32, tag="pu")
                nc.tensor.matmul(po[:, :], lhsT=sut[:, 0:128], rhs=wpsb[:, 0:256], start=True, stop=False)
                nc.tensor.matmul(po[:, :], lhsT=sut[:, 128:256], rhs=wpsb[:, 256:512], start=False, stop=True)
                rd = rdpool.tile([128, 1], f32, tag="rd")
                nc.vector.reciprocal(out=rd[:, :], in_=uf[:, 256:257])
                osb = osbpool.tile([128, 256], f16, tag="osb")
                nc.vector.scalar_tensor_tensor(
                    out=osb[:, :],
                    in0=po[:, :],
                    scalar=rd[:, 0:1],
                    in1=bpb[:, :],
                    op0=Alu.mult,
                    op1=Alu.add,
                )
                nc.sync.dma_start(
                    out=out_sh[g * 128 : (g + 1) * 128, :], in_=osb[:, :]
                )
            if repeats > 1:
                rep_cm.__exit__(None, None, None)
    return nc


def _prepare(x, batch, W1, b1, w2, b2, Wp, bp):
    import ml_dtypes

    _patch_drain()

    x = np.asarray(x, dtype=np.float32)
    batch_np = np.asarray(batch).astype(np.int64)
    W1 = np.asarray(W1, dtype=np.float32)
    b1 = np.asarray(b1, dtype=np.float32)
    w2 = np.asarray(w2, dtype=np.float32)
    b2 = float(np.asarray(b2))
    Wp = np.asarray(Wp, dtype=np.float32)
    bp = np.asarray(bp, dtype=np.float32)

    n, d = x.shape
    assert (n, d) == (N, D)

    # piece p (p = 0..31): nodes whose segment is in [128p, 128(p+1))
    bounds = np.searchsorted(batch_np, np.arange(0, B + 1, CHUNK))  # [33]
    piece_nodes = np.diff(bounds)
    SG = int(np.ceil(piece_nodes.max() / (SPS * SUB)))
    n_super = GROUPS * SG
    n_sub = n_super * SPS
    n_nodes_pad = n_sub * SUB

    nc = _build_nc(SG, ebias_val=b2 - C_OFF)
    import sys
    _m = sys.modules[__name__]
    _m._last_SG = SG
    _m._last_ebias = b2 - C_OFF

    # constant payloads (shared by all cores)
    w1_dt = ml_dtypes.float8_e4m3 if H_FP8 else ml_dtypes.bfloat16
    w1sb = np.zeros((128, 512), dtype=w1_dt)
    for k in range(2):
        for dblk in range(2):
            w1sb[:, (2 * k + dblk) * 128 : (2 * k + dblk + 1) * 128] = (
                W1[k * 128 : (k + 1) * 128, dblk * 128 : (dblk + 1) * 128]
            ).astype(w1_dt)
    b1c = np.stack([b1[0:128], b1[128:256]], axis=1).astype(np.float32)
    w2c = np.stack([w2[0:128], w2[128:256]], axis=1).astype(ml_dtypes.bfloat16)
    wpsb = np.zeros((128, 512), dtype=np.float32)
    wpsb[:, 0:256] = Wp[0:128, :]
    wpsb[:, 256:512] = Wp[128:256, :]
    bpb = np.tile(bp[None, :], (128, 1)).astype(np.float32)
    iota = np.tile(np.arange(128, dtype=np.float32)[None, :], (128, 1))
    idf = np.eye(128, dtype=np.float32)

    in_maps = []
    for c in range(NCORES):
        xflat = np.zeros((n_nodes_pad, D), dtype=np.float32)
        rel_c = np.full(n_sub * SUB, -1.0, dtype=np.float32)
        for g in range(GROUPS):
            p = c * GROUPS + g
            plo, phi = int(bounds[p]), int(bounds[p + 1])
            npc = phi - plo
            off = g * SG * SPS * SUB
            xflat[off : off + npc] = x[plo:phi]
            rel_c[off : off + npc] = (batch_np[plo:phi] - (p * CHUNK)).astype(
                np.float32
            )
        xbf = xflat.astype(ml_dtypes.bfloat16)
        # feature-major: [n_super, 128 part, k*512 + j*128 + n]
        xtb_c = np.ascontiguousarray(
            xflat.astype(w1_dt if H_FP8 else ml_dtypes.bfloat16)
            .reshape(n_super, SPS, SUB, 2, 128).transpose(0, 4, 3, 1, 2)
            .reshape(n_super, SUB, SPS * 256)
        )
        # node-major augmented: [n_super, 128 part, j*258 + (x | 1 1)]
        xb_c = np.ones((n_super, SUB, SPS, 258), dtype=ml_dtypes.bfloat16)
        xb_c[:, :, :, 0:256] = xbf.reshape(n_super, SPS, SUB, D).transpose(
            0, 2, 1, 3
        )
        xb_c = np.ascontiguousarray(xb_c.reshape(n_super, SUB, SPS * 258))
        relT_c = np.ascontiguousarray(
            rel_c.reshape(n_sub, SUB).T
        )  # [128, n_sub]
        in_maps.append(
            {
                "xtb": xtb_c,
                "xb": xb_c,
                "relT": relT_c,
                "w1sb": w1sb,
                "b1c": b1c,
                "w2c": w2c,
                "wpsb": wpsb,
                "bpb": bpb,
                "iota": iota,
                "idf": idf,
                "ebias": np.full((128, 1), b2 - C_OFF, dtype=np.float32),
            }
        )

    return nc, in_maps


def kernel(x, batch, W1, b1, w2, b2, Wp, bp):
    from concourse.bass_utils import run_bass_kernel_spmd

    nc, in_maps = _prepare(x, batch, W1, b1, w2, b2, Wp, bp)
    import kernel as _self
    res = run_bass_kernel_spmd(nc, in_maps, core_ids=list(range(NCORES)))
    _self._last_res = res
    out = np.concatenate([res.results[c]["out"] for c in range(NCORES)], axis=0)
    return out.astype(np.float32)
